# revision 10
# baseline (speedup 1.0000x reference)
"""AttnBlock (GroupNorm + tiny per-sample R*R softmax attention) on 8 trn2 cores.

Data parallel: B=16384 rows split 2048/core; params replicated.
Per core: 16 blocks of 128 samples (samples on partitions).
  - GroupNorm fp32 on DVE/ACT (rstd via Exp(-0.5*Ln(var+eps)) -> single ACT table set)
  - gamma folded into projection weights on host; beta folded into projection biases;
    residual path applies gamma/beta via broadcast tiles on GPSIMD
  - k/q/v projections on PE in bf16 (xn transposed on PE); biases via K=1 ones matmul
  - attention i-loop: ACT Exp(scale=k[:,i]) with fused accum_out denominator;
    DVE tensor_tensor_reduce for the numerator. e_i alternates PSUM-f32 / SBUF-bf16
    to balance ACT vs DVE.
"""

import os
import sys
import math
import numpy as np

try:
    import concourse.bass as bass
except Exception:
    for _p in ("/opt/trn_rl_repo", "/root/.axon_site/_ro/trn_rl_repo"):
        if os.path.isdir(_p) and _p not in sys.path:
            sys.path.insert(0, _p)
    import concourse.bass as bass

import concourse.mybir as mybir
import concourse.tile as tile
import concourse.bacc as bacc
from concourse.bass import AP, MemorySpace
from concourse.bass_utils import run_bass_kernel_spmd
import ml_dtypes

B, C, R, G = 16384, 512, 128, 32
GS = C // G  # 16
NCORES = 8
BLOC = B // NCORES   # 2048
P = 128
NBLK = BLOC // P     # 16
EPS = 1e-6
F32 = mybir.dt.float32
BF16 = mybir.dt.bfloat16
AF = mybir.ActivationFunctionType
OP = mybir.AluOpType
BF16NP = ml_dtypes.bfloat16

LAST_RESULTS = None

# fraction of i-iterations whose e_i goes to PSUM (f32) vs SBUF (bf16)
SBUF_EVERY = 4  # i % SBUF_EVERY == 0 -> SBUF bf16 path


def _bc(ap: AP, n: int) -> AP:
    """Append an inner broadcast (step 0) dim of size n to an AP."""
    return AP(ap.tensor, ap.offset, list(ap.ap) + [[0, n]])


def _build_nc():
    nc = bacc.Bacc()

    x = nc.dram_tensor("x", (BLOC, C), F32, kind="ExternalInput")
    kwT = nc.dram_tensor("kwT", (C, R), BF16, kind="ExternalInput")
    qwT = nc.dram_tensor("qwT", (C, R), BF16, kind="ExternalInput")
    vwT = nc.dram_tensor("vwT", (C, R), BF16, kind="ExternalInput")
    rwT = nc.dram_tensor("rwT", (R, C), BF16, kind="ExternalInput")
    kb = nc.dram_tensor("kb", (1, R), F32, kind="ExternalInput")
    qb = nc.dram_tensor("qb", (1, R), F32, kind="ExternalInput")
    vb = nc.dram_tensor("vb", (1, R), F32, kind="ExternalInput")
    gamma_b = nc.dram_tensor("gamma_b", (P, C), F32, kind="ExternalInput")
    betarb_b = nc.dram_tensor("betarb_b", (P, C), F32, kind="ExternalInput")
    ident = nc.dram_tensor("ident", (P, P), BF16, kind="ExternalInput")
    out = nc.dram_tensor("out", (BLOC, C), F32, kind="ExternalOutput")

    xb = x.rearrange("(n p) c -> n p c", p=P)
    ob = out.rearrange("(n p) c -> n p c", p=P)

    inv_sqrt_r = 1.0 / math.sqrt(R)

    from contextlib import ExitStack
    with ExitStack() as es:
        tc = es.enter_context(tile.TileContext(nc))
        pool = lambda *a, **kw: es.enter_context(tc.tile_pool(*a, **kw))
        const = pool(name="const", bufs=1)
        pio = pool(name="pio", bufs=3)
        pgn = pool(name="pgn", bufs=2)
        pst = pool(name="pst", bufs=4)
        pz = pool(name="pz", bufs=3)
        pzb = pool(name="pzb", bufs=2)
        pzT = pool(name="pzT", bufs=6)
        pksb = pool(name="pksb", bufs=2)
        pnd = pool(name="pnd", bufs=3)
        pes = pool(name="pes", bufs=3)
        pjk = pool(name="pjk", bufs=3)
        pat = pool(name="pat", bufs=2)
        po2s = pool(name="po2s", bufs=2)
        pres = pool(name="pres", bufs=3)
        # PSUM pools: each tile occupies one 2KB bank; total must be <= 8
        pq = pool(name="pq", bufs=2, space=MemorySpace.PSUM)
        pkv = pool(name="pkv", bufs=1, space=MemorySpace.PSUM)
        pep = pool(name="pep", bufs=2, space=MemorySpace.PSUM)
        ptr = pool(name="ptr", bufs=1, space=MemorySpace.PSUM)
        po2 = pool(name="po2", bufs=1, space=MemorySpace.PSUM)
        if True:
            # ---- one-time constants ----
            t_kwT = const.tile([P, 4, R], BF16, tag="kwT")
            t_qwT = const.tile([P, 4, R], BF16, tag="qwT")
            t_vwT = const.tile([P, 4, R], BF16, tag="vwT")
            t_rwT = const.tile([P, C], BF16, tag="rwT")
            t_kb = const.tile([1, R], F32, tag="kb")
            t_qb = const.tile([1, R], F32, tag="qb")
            t_vb = const.tile([1, R], F32, tag="vb")
            t_gam = const.tile([P, C], F32, tag="gam")
            t_brb = const.tile([P, C], F32, tag="brb")
            t_id = const.tile([P, P], BF16, tag="id")
            t_ones = const.tile([1, P], F32, tag="ones")
            t_eps = const.tile([P, 1], F32, tag="eps")
            nc.vector.memset(t_eps[:], EPS)

            nc.sync.dma_start(t_kwT[:], kwT.rearrange("(n p) r -> p n r", p=P))
            nc.sync.dma_start(t_qwT[:], qwT.rearrange("(n p) r -> p n r", p=P))
            nc.sync.dma_start(t_vwT[:], vwT.rearrange("(n p) r -> p n r", p=P))
            nc.sync.dma_start(t_rwT[:], rwT[:])
            nc.sync.dma_start(t_kb[:], kb[:])
            nc.sync.dma_start(t_qb[:], qb[:])
            nc.sync.dma_start(t_vb[:], vb[:])
            nc.sync.dma_start(t_gam[:], gamma_b[:])
            nc.sync.dma_start(t_brb[:], betarb_b[:])
            nc.sync.dma_start(t_id[:], ident[:])
            nc.vector.memset(t_ones[:], 1.0)

            for n in range(NBLK):
                # ---- load ----
                xt = pio.tile([P, C], F32, tag="xin")
                nc.sync.dma_start(xt[:], xb[n])

                # ---- GroupNorm stats (fp32) ----
                sq = pgn.tile([P, C], F32, tag="sq")
                nc.scalar.activation(sq[:], xt[:], AF.Square)
                x3 = xt[:].rearrange("p (g s) -> p g s", s=GS)
                sq3 = sq[:].rearrange("p (g s) -> p g s", s=GS)
                s1 = pst.tile([P, G], F32, tag="s1")
                s2 = pst.tile([P, G], F32, tag="s2")
                nc.vector.tensor_reduce(s1[:], x3, axis=mybir.AxisListType.X, op=OP.add)
                nc.vector.tensor_reduce(s2[:], sq3, axis=mybir.AxisListType.X, op=OP.add)
                mean = pst.tile([P, G], F32, tag="mean")
                nc.vector.tensor_scalar_mul(mean[:], s1[:], 1.0 / GS)
                m2 = pst.tile([P, G], F32, tag="m2")
                nc.vector.tensor_tensor(m2[:], mean[:], mean[:], op=OP.mult)
                var = pst.tile([P, G], F32, tag="var")
                nc.vector.tensor_scalar_mul(var[:], s2[:], 1.0 / GS)
                nc.vector.tensor_tensor(var[:], var[:], m2[:], op=OP.subtract)
                lnv = pst.tile([P, G], F32, tag="lnv")
                nc.scalar.activation(lnv[:], var[:], AF.Ln, bias=t_eps[:])
                rstd = pst.tile([P, G], F32, tag="rstd")
                nc.scalar.activation(rstd[:], lnv[:], AF.Exp, scale=-0.5)

                # ---- normalize: z = (x - mean) * rstd  (per 16-ch group) ----
                z = pz.tile([P, C], F32, tag="z")
                z3 = z[:].rearrange("p (g s) -> p g s", s=GS)
                nc.vector.tensor_tensor(z3, x3, _bc(mean[:], GS), op=OP.subtract)
                nc.vector.tensor_tensor(z3, z3, _bc(rstd[:], GS), op=OP.mult)

                # ---- cast to bf16 + transpose 4 chunks on PE ----
                zb = pzb.tile([P, C], BF16, tag="zb")
                nc.vector.tensor_copy(zb[:], z[:])
                zTs = []
                for ch in range(4):
                    tp = ptr.tile([P, P], BF16, tag="tr")
                    nc.tensor.transpose(tp[:], zb[:, ch * P:(ch + 1) * P], t_id[:])
                    zT = pzT.tile([P, P], BF16, tag="zT")
                    if ch % 2 == 0:
                        nc.scalar.copy(zT[:], tp[:])
                    else:
                        nc.vector.tensor_copy(zT[:], tp[:])
                    zTs.append(zT)

                # ---- projections on PE (bf16), bias via K=1 ones matmul (fp32) ----
                kps = pkv.tile([P, R], F32, tag="kps")
                qps = pq.tile([P, R], F32, tag="qps")
                vps = pkv.tile([P, R], F32, tag="vps")
                for ch in range(4):
                    nc.tensor.matmul(kps[:], zTs[ch][:], t_kwT[:, ch, :],
                                     start=(ch == 0), stop=False)
                for ch in range(4):
                    nc.tensor.matmul(qps[:], zTs[ch][:], t_qwT[:, ch, :],
                                     start=(ch == 0), stop=False)
                for ch in range(4):
                    nc.tensor.matmul(vps[:], zTs[ch][:], t_vwT[:, ch, :],
                                     start=(ch == 0), stop=False)
                nc.tensor.matmul(kps[:], t_ones[:], t_kb[:], start=False, stop=True)
                nc.tensor.matmul(qps[:], t_ones[:], t_qb[:], start=False, stop=True)
                nc.tensor.matmul(vps[:], t_ones[:], t_vb[:], start=False, stop=True)

                # k to SBUF (scale AP source), v to SBUF in both f32 and bf16
                ksb = pksb.tile([P, R], F32, tag="ksb")
                nc.scalar.copy(ksb[:], kps[:])
                vf = pksb.tile([P, R], F32, tag="vf")
                nc.vector.tensor_copy(vf[:], vps[:])
                vb16 = pksb.tile([P, R], BF16, tag="vb16")
                nc.scalar.copy(vb16[:], vps[:])

                # ---- attention i-loop ----
                numer = pnd.tile([P, R], F32, tag="numer")
                denom = pnd.tile([P, R], F32, tag="denom")
                for i in range(R):
                    kcol = ksb[:, i:i + 1]
                    if i % SBUF_EVERY == 0:
                        e = pes.tile([P, R], BF16, tag="es")
                        nc.scalar.activation(e[:], qps[:], AF.Exp, scale=kcol,
                                             accum_out=denom[:, i:i + 1])
                        jk = pjk.tile([P, R], BF16, tag="jks")
                        nc.vector.affine_mul_reduce(
                            out=jk[:], accum_out=numer[:, i:i + 1],
                            in0=e[:], in1=vb16[:], scale=1.0, bias=0.0)
                    else:
                        e = pep.tile([P, R], F32, tag="ep")
                        nc.scalar.activation(e[:], qps[:], AF.Exp, scale=kcol,
                                             accum_out=denom[:, i:i + 1])
                        jk = pjk.tile([P, R], F32, tag="jkp")
                        nc.vector.affine_mul_reduce(
                            out=jk[:], accum_out=numer[:, i:i + 1],
                            in0=e[:], in1=vf[:], scale=1.0, bias=0.0)

                # ---- attn = numer/denom * 1/sqrt(R), cast bf16 ----
                rec = pnd.tile([P, R], F32, tag="rec")
                nc.vector.reciprocal_approx_fast(rec[:], denom[:])
                attn = pat.tile([P, R], BF16, tag="attn")
                ja = pat.tile([P, 1], F32, tag="ja")
                nc.vector.affine_mul_reduce(
                    out=attn[:], accum_out=ja[:], in0=numer[:], in1=rec[:],
                    scale=inv_sqrt_r, bias=0.0)

                # ---- transpose attn, out2 = attnT.T @ rwT ----
                tpa = ptr.tile([P, P], BF16, tag="tr")
                nc.tensor.transpose(tpa[:], attn[:], t_id[:])
                aT = pat.tile([P, P], BF16, tag="aT")
                nc.vector.tensor_copy(aT[:], tpa[:])
                o2 = po2.tile([P, C], F32, tag="o2")
                nc.tensor.matmul(o2[:], aT[:], t_rwT[:], start=True, stop=True)
                o2s = po2s.tile([P, C], F32, tag="o2s")
                nc.scalar.copy(o2s[:], o2[:])

                # ---- epilogue on GPSIMD: final = z*gamma + (beta + r_b) + out2 ----
                r1 = pres.tile([P, C], F32, tag="r1")
                nc.gpsimd.tensor_tensor(r1[:], z[:], t_gam[:], op=OP.mult)
                r2 = pres.tile([P, C], F32, tag="r2")
                nc.gpsimd.tensor_tensor(r2[:], r1[:], t_brb[:], op=OP.add)
                fin = pres.tile([P, C], F32, tag="fin")
                nc.gpsimd.tensor_tensor(fin[:], r2[:], o2s[:], op=OP.add)
                nc.sync.dma_start(ob[n], fin[:])

    nc.compile()
    return nc


_NC = None


def kernel(x, gn_gamma, gn_beta, k_w, k_b, q_w, q_b, v_w, v_b, r_w, r_b):
    global _NC, LAST_RESULTS
    x = np.asarray(x, np.float32)
    gn_gamma = np.asarray(gn_gamma, np.float32)
    gn_beta = np.asarray(gn_beta, np.float32)

    # host-side folding: xn = z*gamma + beta  =>  proj(xn) = z @ (w*gamma).T + (w@beta + b)
    def fold(w, b):
        wT = np.ascontiguousarray((np.asarray(w, np.float32) * gn_gamma[None, :]).T)
        be = (np.asarray(b, np.float32) + np.asarray(w, np.float32) @ gn_beta)
        return wT.astype(BF16NP), np.ascontiguousarray(be[None, :].astype(np.float32))

    kwT, kb = fold(k_w, k_b)
    qwT, qb = fold(q_w, q_b)
    vwT, vb = fold(v_w, v_b)
    rwT = np.ascontiguousarray(np.asarray(r_w, np.float32).T).astype(BF16NP)
    gamma_b = np.ascontiguousarray(np.broadcast_to(gn_gamma, (P, C))).astype(np.float32)
    betarb = gn_beta + np.asarray(r_b, np.float32)
    betarb_b = np.ascontiguousarray(np.broadcast_to(betarb, (P, C))).astype(np.float32)
    ident = np.eye(P, dtype=np.float32).astype(BF16NP)

    shared = dict(kwT=kwT, qwT=qwT, vwT=vwT, rwT=rwT, kb=kb, qb=qb, vb=vb,
                  gamma_b=gamma_b, betarb_b=betarb_b, ident=ident)
    in_maps = [dict(x=np.ascontiguousarray(x[c * BLOC:(c + 1) * BLOC]), **shared)
               for c in range(NCORES)]

    if _NC is None:
        _NC = _build_nc()
    res = run_bass_kernel_spmd(_NC, in_maps, list(range(NCORES)))
    LAST_RESULTS = res
    return np.concatenate([np.asarray(res.results[i]["out"]) for i in range(NCORES)],
                          axis=0)
